# revision 19
# baseline (speedup 1.0000x reference)
"""Trainium2 Bass kernel for MultiHeadPosAttn (attention + BN + FFN + BN).

Sharding: data-parallel over batch across 8 NeuronCores (2 images/core).
BatchNorm batch statistics are combined with a tiny (2KB) AllReduce.

Math notes (verified exactly equivalent to the reference):
  - bk cancels in softmax (adds a per-query constant to every logit row).
  - bv cancels in BN1 (per-channel constant shift; softmax rows sum to 1).
  - b2 cancels in BN2 (per-channel constant shift).
  - PReLU(y) = Lrelu(y) with alpha = a (ACT supports a slope parameter).
  - softmax needs no max-subtraction: |logits| <= ~66 so exp() stays in
    fp32 range (max ~3e28 << 3.4e38).
Softmax denominator comes from an extra all-ones column in each head's
V^T block, so the attention matmul also produces sum_k(P) per query.
The V^T block for head h occupies lhsT columns so that the head's
output lands directly on its target partitions (even heads: d at
cols 0-63 + ones at col 64; odd heads: ones at col 63 + d at cols
64-127), avoiding any partition-shifting DMA.  The denominator row is
broadcast across partitions with a GPSIMD partition_broadcast (no DRAM
round trip).
"""

import numpy as np

import concourse.bass as bass
import concourse.bacc as bacc
import concourse.tile as tile
from concourse import mybir
from concourse import bass_utils

F32 = mybir.dt.float32
BF16 = mybir.dt.bfloat16
F16 = mybir.dt.float16

B, C, HH, WW = 16, 256, 32, 32
N = HH * WW              # 1024 spatial positions
NH, DH = 4, 64           # heads, head dim
DFF = 4 * C              # 1024
EPS = 1e-5
NCORES = 8
BL = B // NCORES         # 2 images per core
NCH = C // 128           # 2 channel chunks of 128
NFC = DFF // 128         # 8 ffn chunks
NNC = N // 128           # 8 position chunks


def _build(a_slope: float):
    nc = bacc.Bacc("TRN2", target_bir_lowering=False, debug=False,
                   num_devices=NCORES)

    # all big inputs are pre-packed host-side into [128, free] partition-major
    # layouts so every load DMA is 128 descriptors of >=512B contiguous rows
    x_d = nc.dram_tensor("x", [BL, 128, NCH * N], F16, kind="ExternalInput")
    wq_d = nc.dram_tensor("wqT", [128, NCH * C], F16, kind="ExternalInput")
    wk_d = nc.dram_tensor("wkT", [128, NCH * C], F16, kind="ExternalInput")
    wv_d = nc.dram_tensor("wvT", [128, NCH * NH * DH], F16, kind="ExternalInput")
    w1_d = nc.dram_tensor("w1T", [128, NCH * DFF], F16, kind="ExternalInput")
    w2_d = nc.dram_tensor("w2T", [128, NFC * C], F16, kind="ExternalInput")
    # bq(2) | b1(8) | gamma(2) | beta(2) packed per partition
    sm_d = nc.dram_tensor("sm", [128, 14], F32, kind="ExternalInput")
    out_d = nc.dram_tensor("out", [BL, C, N], F32, kind="ExternalOutput")

    with tile.TileContext(nc) as tc:
        _emit(tc, a_slope,
              x_d=x_d, wq_d=wq_d, wk_d=wk_d, wv_d=wv_d,
              w1_d=w1_d, w2_d=w2_d, sm_d=sm_d, out_d=out_d)
    nc.compile()
    return nc


def _emit(tc, a_slope, *, x_d, wq_d, wk_d, wv_d, w1_d, w2_d, sm_d, out_d):
    nc = tc.nc
    from contextlib import ExitStack

    ctx = ExitStack()
    with ctx:
        const = ctx.enter_context(tc.tile_pool(name="const", bufs=1))
        data = ctx.enter_context(tc.tile_pool(name="data", bufs=1))
        work = ctx.enter_context(tc.tile_pool(name="work", bufs=1))
        dram = ctx.enter_context(tc.tile_pool(name="dram", bufs=1, space="DRAM"))

        # ---- loads, spread across engines so the QKV-critical tensors
        # (wq, x0, wk, wv) land ASAP; FFN weights queue behind ----
        xs = []
        for img in range(BL):
            xs.append(data.tile([128, NCH, N], F16, name=f"xs{img}",
                                tag=f"xs{img}"))
        wq_sb = const.tile([128, NCH, C], F16, name="wq_sb")
        wk_sb = const.tile([128, NCH, C], F16, name="wk_sb")
        wv_sb = const.tile([128, NCH, NH * DH], F16, name="wv_sb")
        w1_sb = const.tile([128, NCH, DFF], F16, name="w1_sb")
        w2_sb = const.tile([128, NFC, C], F16, name="w2_sb")

        # scalar engine: wq then x0 (first Q matmul needs both), smalls
        nc.scalar.dma_start(out=wq_sb,
                            in_=wq_d.ap().rearrange("p (k m) -> p k m", m=C))
        nc.scalar.dma_start(out=xs[0],
                            in_=x_d.ap()[0].rearrange("p (c n) -> p c n", n=N))
        sm_sb = const.tile([128, 14], F32, name="sm_sb")
        nc.scalar.dma_start(out=sm_sb, in_=sm_d.ap())
        bq_sb = sm_sb[:, 0:NCH]
        b1_sb = sm_sb[:, NCH:NCH + NFC]
        gam_sb = sm_sb[:, NCH + NFC:NCH + NFC + NCH]
        bet_sb = sm_sb[:, NCH + NFC + NCH:NCH + NFC + 2 * NCH]
        # sync engine: wk, wv, x1
        nc.sync.dma_start(out=wk_sb,
                          in_=wk_d.ap().rearrange("p (k m) -> p k m", m=C))
        nc.sync.dma_start(out=wv_sb,
                          in_=wv_d.ap().rearrange("p (k m) -> p k m", m=NH * DH))
        nc.sync.dma_start(out=xs[1],
                          in_=x_d.ap()[1].rearrange("p (c n) -> p c n", n=N))
        # gpsimd: FFN weights (not needed until after attention)
        nc.gpsimd.dma_start(out=w1_sb,
                            in_=w1_d.ap().rearrange("p (k m) -> p k m", m=DFF))
        nc.gpsimd.dma_start(out=w2_sb,
                            in_=w2_d.ap().rearrange("p (k m) -> p k m", m=C))

        # PE warm-up: ~60 tiny matmuls straight after the preamble keep the
        # HAM activity window busy so QKV starts at 2.4GHz instead of 1.2.
        wrm_t = const.tile([128, 128], F16, name="wrm_t")
        nc.vector.memset(wrm_t, 0.5)
        warm_sb = const.tile([1, 64], F32, name="warm_sb")
        nc.vector.memset(warm_sb, 0.0)
        with tc.tile_pool(name="wrps", bufs=1, space="PSUM") as wrps:
            wp_t = wrps.tile([128, 128], F32, name="wp_t")
            for _ in range(60):
                nc.tensor.matmul(wp_t, lhsT=wrm_t, rhs=wrm_t,
                                 start=True, stop=True)
            # keep the dummies alive: route one lane into the warm payload
            nc.vector.tensor_copy(warm_sb[0:1, 63:64], wp_t[0:1, 0:1])
        for wi in range(2):
            w_in = dram.tile([64], F32, name=f"warm{wi}_in", tag=f"warm{wi}_in")
            w_out = dram.tile([64], F32, name=f"warm{wi}_out",
                              tag=f"warm{wi}_out", addr_space="Shared")
            nc.sync.dma_start(out=w_in.unsqueeze(0), in_=warm_sb)
            nc.gpsimd.collective_compute(
                "AllReduce", mybir.AluOpType.add,
                replica_groups=[list(range(NCORES))],
                ins=[w_in.opt()], outs=[w_out.opt()])

        # ---- persistent SBUF tensors ----
        q_sb, k_sb, vt_sb, o_sb, mh_sb, u_sb = [], [], [], [], [], []
        for img in range(BL):
            q_sb.append(data.tile([128, NCH, N], F16, name=f"q{img}", tag=f"q{img}"))
            k_sb.append(data.tile([128, NCH, N], F16, name=f"k{img}", tag=f"k{img}"))
            vt_sb.append(data.tile([128, NNC, NH * 128], BF16, name=f"vt{img}",
                                   tag=f"vt{img}"))
            o_sb.append(data.tile([128, NCH, N], F32, name=f"o{img}", tag=f"o{img}"))
            mh_sb.append(data.tile([128, NCH, N], F16, name=f"mh{img}",
                                   tag=f"mh{img}"))
            u_sb.append(data.tile([128, NCH, N], F32, name=f"u{img}", tag=f"u{img}"))

        # V^T layout per head block (128 cols): even heads [v(64) | 1 | 0*63],
        # odd heads [1 | 0*63 | v(64)] -- the ones (denominator) column must
        # land on a 32-aligned PSUM partition (0 or 64).
        for img in range(BL):
            vt4 = vt_sb[img].rearrange("p a (h d) -> p a h d", d=128)
            for h in range(NH):
                if h % 2 == 0:
                    nc.gpsimd.memset(vt4[:, :, h, DH + 1:128], 0.0)
                    nc.gpsimd.memset(vt4[:, :, h, DH:DH + 1], 1.0)
                else:
                    nc.gpsimd.memset(vt4[:, :, h, 1:DH], 0.0)
                    nc.gpsimd.memset(vt4[:, :, h, 0:1], 1.0)

        st1 = work.tile([128, NCH, BL * 2, 6], F32, name="bn1_stats",
                        tag="bn1_stats")
        st2 = work.tile([128, NCH, BL * 2, 6], F32, name="bn2_stats",
                        tag="bn2_stats")

        # =========== per image: QKV (own pools) then heads (own pools) ====
        def make_qkv(qkps, vtps):
            def emit_qkv_q(img):
                for mc in range(NCH):
                    qp = qkps.tile([128, N], F32, tag="qp", bufs=2)
                    for kc in range(NCH):
                        for mv in range(2):
                            nc.tensor.matmul(
                                qp[:, mv * 512:(mv + 1) * 512],
                                lhsT=(wq_sb[:, kc, mc * 128:(mc + 1) * 128]),
                                rhs=(xs[img][:, kc, mv * 512:(mv + 1) * 512]),
                                start=(kc == 0), stop=(kc == NCH - 1))
                    nc.scalar.activation(q_sb[img][:, mc, :], qp,
                                         mybir.ActivationFunctionType.Identity,
                                         bias=bq_sb[:, mc:mc + 1])

            def emit_qkv_k(img):
                for mc in range(NCH):
                    kp = qkps.tile([128, N], F32, tag="qp", bufs=2)
                    for kc in range(NCH):
                        for mv in range(2):
                            nc.tensor.matmul(
                                kp[:, mv * 512:(mv + 1) * 512],
                                lhsT=(wk_sb[:, kc, mc * 128:(mc + 1) * 128]),
                                rhs=(xs[img][:, kc, mv * 512:(mv + 1) * 512]),
                                start=(kc == 0), stop=(kc == NCH - 1))
                    nc.scalar.activation(k_sb[img][:, mc, :], kp,
                                         mybir.ActivationFunctionType.Identity)

            def emit_qkv_v(img, pcs):
                vt4 = vt_sb[img].rearrange("p a (h d) -> p a h d", d=128)
                for pc in pcs:
                    vp = vtps.tile([128, N], F32, tag="vp", bufs=2)
                    for kc in range(NCH):
                        nc.tensor.matmul(
                            vp[:, 0:NH * DH],
                            lhsT=(xs[img][:, kc, pc * 128:(pc + 1) * 128]),
                            rhs=(wv_sb[:, kc, :]),
                            start=(kc == 0), stop=(kc == NCH - 1))
                    for h in range(NH):
                        dst0 = 0 if h % 2 == 0 else 64
                        nc.vector.tensor_copy(
                            vt4[:, pc, h, dst0:dst0 + DH],
                            vp[:, h * DH:(h + 1) * DH])

            return emit_qkv_q, emit_qkv_k, emit_qkv_v

        def make_head(etps, oaps):
            def emit_head(img, h):
                hc, ho = h // 2, (h % 2) * 64
                denp = 64 if h % 2 == 0 else 0
                q_h = q_sb[img][ho:ho + 64, hc, :]
                k_h = k_sb[img][ho:ho + 64, hc, :]
                oaug = oaps.tile([128, N], F32, tag="oaug", bufs=2)
                for pc in range(NNC):
                    et = etps.tile([128, N], F32, tag="et", bufs=2)
                    for mv in range(2):
                        nc.tensor.matmul(
                            et[:, mv * 512:(mv + 1) * 512],
                            lhsT=(k_h[:, pc * 128:(pc + 1) * 128]),
                            rhs=(q_h[:, mv * 512:(mv + 1) * 512]),
                            start=True, stop=True)
                    p_t = work.tile([128, N], BF16, name="p_t", tag="p_t", bufs=6)
                    nc.scalar.activation(p_t, et,
                                         mybir.ActivationFunctionType.Exp)
                    for mv in range(2):
                        nc.tensor.matmul(
                            oaug[:, mv * 512:(mv + 1) * 512],
                            lhsT=(vt_sb[img][:, pc, h * 128:(h + 1) * 128]),
                            rhs=(p_t[:, mv * 512:(mv + 1) * 512]),
                            start=(pc == 0), stop=(pc == NNC - 1))
                # softmax denominator: row `denp` of oaug.  Copy to SBUF,
                # broadcast across all partitions on GPSIMD, reciprocal
                # (base-0 custom DVE op), then scale the head's 64 rows.
                # For the final head the chain is split into halves so the
                # BN1 stats (and the AllReduce behind them) start sooner.
                tail = (img == BL - 1 and h == NH - 1)
                halves = ((0, 512), (512, 1024)) if tail else ((0, 1024),)
                dsb = work.tile([128, N], F32, name="dsb", tag="dsb", bufs=2)
                dbc = work.tile([128, N], F32, name="dbc", tag="dbc", bufs=2)
                rbc = work.tile([128, N], F32, name="rbc", tag="rbc", bufs=2)
                dsb0 = None
                for lo, hi in halves:
                    nc.vector.tensor_copy(dsb[denp:denp + 1, lo:hi],
                                          oaug[denp:denp + 1, lo:hi])
                    srct = dsb
                    if denp != 0:
                        # partition_broadcast reads ABSOLUTE partition 0 on
                        # HW: bounce the row down with a small SBUF DMA.
                        if dsb0 is None:
                            dsb0 = work.tile([128, N], F32, name="dsb0",
                                             tag="dsb0", bufs=2)
                        nc.gpsimd.dma_start(out=dsb0[0:1, lo:hi],
                                            in_=dsb[denp:denp + 1, lo:hi])
                        srct = dsb0
                    nc.gpsimd.partition_broadcast(dbc[:, lo:hi],
                                                  srct[0:1, lo:hi])
                    nc.vector.reciprocal_approx_fast(out=rbc[:, lo:hi],
                                                     in_=dbc[:, lo:hi])
                    nc.vector.tensor_mul(o_sb[img][ho:ho + 64, hc, lo:hi],
                                         oaug[ho:ho + 64, lo:hi],
                                         rbc[ho:ho + 64, lo:hi])
                    if h % 2 == 1 and tail:
                        sg = lo // 512
                        nc.vector.tensor_add(
                            o_sb[img][:, hc, lo:hi],
                            o_sb[img][:, hc, lo:hi], xs[img][:, hc, lo:hi])
                        nc.vector.bn_stats(
                            out=st1[:, hc, img * 2 + sg, :],
                            in_=o_sb[img][:, hc, lo:hi])
                if h % 2 == 1 and not tail:
                    # both heads of chunk hc done -> residual + local stats
                    nc.vector.tensor_add(o_sb[img][:, hc, :],
                                         o_sb[img][:, hc, :],
                                         xs[img][:, hc, :])
                    for sg in range(2):
                        nc.vector.bn_stats(
                            out=st1[:, hc, img * 2 + sg, :],
                            in_=o_sb[img][:, hc, sg * 512:(sg + 1) * 512])

            return emit_head

        for img in range(BL):
            with tc.tile_pool(name=f"qkps{img}", bufs=2, space="PSUM") as qkps, \
                 tc.tile_pool(name=f"vtps{img}", bufs=2, space="PSUM") as vtps:
                eq, ek, ev = make_qkv(qkps, vtps)
                eq(img)
                ek(img)
                ev(img, range(NNC))
            with tc.tile_pool(name=f"etps{img}", bufs=2, space="PSUM") as etps, \
                 tc.tile_pool(name=f"oaps{img}", bufs=2, space="PSUM") as oaps:
                eh = make_head(etps, oaps)
                for h in range(NH):
                    eh(img, h)

        # =========== BN1 ===========
        s1_sb = work.tile([128, NCH], F32, name="s1_sb", tag="bns")
        t1_sb = work.tile([128, NCH], F32, name="t1_sb", tag="bnt")
        eps_unused = None
        cc1 = _bn_allreduce(tc, nc, work, dram, "bn1", st1)
        _bn_finish(tc, nc, work, "bn1", cc_out=cc1, gam_sb=gam_sb,
                   bet_sb=bet_sb, scale_out=s1_sb, shift_out=t1_sb)
        # apply: mh = s*(o+x) + t ; split across ACT (ch0) and DVE (ch1)
        for img in range(BL):
            nc.scalar.activation(mh_sb[img][:, 0, :], o_sb[img][:, 0, :],
                                 mybir.ActivationFunctionType.Identity,
                                 bias=t1_sb[:, 0:1], scale=s1_sb[:, 0:1])
            nc.vector.tensor_scalar(
                out=mh_sb[img][:, 1, :], in0=o_sb[img][:, 1, :],
                scalar1=s1_sb[:, 1:2], scalar2=t1_sb[:, 1:2],
                op0=mybir.AluOpType.mult, op1=mybir.AluOpType.add)

        # =========== FFN (mc-major W2 so stats start early) ===========
        ffs = [[work.tile([128, N], F16, name=f"ffs{img}_{fc}",
                          tag=f"ffs{fc}", bufs=2) for fc in range(NFC)]
               for img in range(BL)]
        with tc.tile_pool(name="ffps", bufs=2, space="PSUM") as ffps, \
             tc.tile_pool(name="ops2", bufs=2, space="PSUM") as ops2:
            def emit_w1(img, fc):
                # W1 matmul + PReLU for one ffn chunk (ACT Prelu honors
                # alpha; Lrelu ignores it on this HW)
                fp = ffps.tile([128, N], F32, tag="fp", bufs=2)
                for kc in range(NCH):
                    for mv in range(2):
                        nc.tensor.matmul(
                            fp[:, mv * 512:(mv + 1) * 512],
                            lhsT=(w1_sb[:, kc, fc * 128:(fc + 1) * 128]),
                            rhs=(mh_sb[img][:, kc, mv * 512:(mv + 1) * 512]),
                            start=(kc == 0), stop=(kc == NCH - 1))
                nc.scalar.activation(
                    ffs[img][fc], fp,
                    mybir.ActivationFunctionType.Prelu,
                    bias=b1_sb[:, fc:fc + 1], alpha=a_slope)

            for img in range(BL):
                for mc in range(NCH):
                    outp = ops2.tile([128, N], F32, tag="outp", bufs=2)
                    if mc == 0:
                        emit_w1(img, 0)
                    for fc in range(NFC):
                        # software pipeline: next chunk's W1 ahead of this
                        # chunk's W2 so PE never stalls on the Prelu
                        if mc == 0 and fc + 1 < NFC:
                            emit_w1(img, fc + 1)
                        for mv in range(2):
                            nc.tensor.matmul(
                                outp[:, mv * 512:(mv + 1) * 512],
                                lhsT=(w2_sb[:, fc, mc * 128:(mc + 1) * 128]),
                                rhs=(ffs[img][fc][:, mv * 512:(mv + 1) * 512]),
                                start=(fc == 0), stop=(fc == NFC - 1))
                    nc.vector.tensor_add(u_sb[img][:, mc, :], outp,
                                         mh_sb[img][:, mc, :])
                    for sg in range(2):
                        nc.vector.bn_stats(
                            out=st2[:, mc, img * 2 + sg, :],
                            in_=u_sb[img][:, mc, sg * 512:(sg + 1) * 512])

        # =========== BN2 + output ===========
        s2_sb = work.tile([128, NCH], F32, name="s2_sb", tag="bns2")
        t2_sb = work.tile([128, NCH], F32, name="t2_sb", tag="bnt2")
        cc2 = _bn_allreduce(tc, nc, work, dram, "bn2", st2)
        _bn_finish(tc, nc, work, "bn2", cc_out=cc2, gam_sb=gam_sb,
                   bet_sb=bet_sb, scale_out=s2_sb, shift_out=t2_sb)
        for img in range(BL):
            outr = out_d.ap()[img].rearrange("(c p) n -> p c n", p=128)
            # ch0 apply on ACT, ch1 on DVE, in half-chunks so the output DMAs
            # start as early as possible (o_sb is dead -> reuse as staging)
            for mv in range(2):
                sl = slice(mv * 512, (mv + 1) * 512)
                nc.scalar.activation(o_sb[img][:, 0, sl], u_sb[img][:, 0, sl],
                                     mybir.ActivationFunctionType.Identity,
                                     bias=t2_sb[:, 0:1], scale=s2_sb[:, 0:1])
                nc.sync.dma_start(out=outr[:, 0, sl], in_=o_sb[img][:, 0, sl])
                nc.vector.tensor_scalar(
                    out=o_sb[img][:, 1, sl], in0=u_sb[img][:, 1, sl],
                    scalar1=s2_sb[:, 1:2], scalar2=t2_sb[:, 1:2],
                    op0=mybir.AluOpType.mult, op1=mybir.AluOpType.add)
                nc.gpsimd.dma_start(out=outr[:, 1, sl], in_=o_sb[img][:, 1, sl])


def _bn_allreduce(tc, nc, work, dram, name, stats):
    """Aggregate all local bn_stats, pack [mean, msq], one 2KB AllReduce."""
    mv_t = work.tile([128, NCH, 2], F32, name=f"{name}_mv", tag=f"{name}_mv")
    pk = work.tile([128, NCH, 2], F32, name=f"{name}_pk", tag=f"{name}_pk")
    for ch in range(NCH):
        nc.vector.bn_aggr(out=mv_t[:, ch, :], in_=stats[:, ch, :, :])
    nc.vector.tensor_mul(pk[:, :, 0:1], mv_t[:, :, 0:1], mv_t[:, :, 0:1])
    nc.vector.tensor_add(pk[:, :, 1:2], mv_t[:, :, 1:2], pk[:, :, 0:1])
    nc.vector.tensor_copy(pk[:, :, 0:1], mv_t[:, :, 0:1])
    cc_in = dram.tile([128 * NCH * 2], F32, name=f"{name}_cc_in",
                      tag=f"{name}_cc_in")
    cc_out = dram.tile([128 * NCH * 2], F32, name=f"{name}_cc_out",
                       tag=f"{name}_cc_out", addr_space="Shared")
    nc.sync.dma_start(out=cc_in.rearrange("(p k) -> p k", p=128), in_=pk)
    nc.gpsimd.collective_compute(
        "AllReduce", mybir.AluOpType.add,
        replica_groups=[list(range(NCORES))],
        ins=[cc_in.opt()], outs=[cc_out.opt()])
    return cc_out


def _bn_finish(tc, nc, work, name, *, cc_out, gam_sb, bet_sb,
               scale_out, shift_out):
    """Turn the AllReduced [mean, msq] sums into per-channel scale/shift.
    rsqrt is DVE-only (bit-trick seed + Newton) to avoid an ACT table
    switch on the critical path."""
    sg_t = work.tile([128, NCH, 2], F32, name=f"{name}_sg", tag=f"{name}_sg")
    nc.sync.dma_start(out=sg_t, in_=cc_out.rearrange("(p k) -> p k", p=128))
    g8 = work.tile([128, NCH, 2], F32, name=f"{name}_g8", tag=f"{name}_g8")
    nc.vector.tensor_scalar_mul(g8, sg_t, 1.0 / NCORES)
    # var = msq - mean^2 + eps   (both channel chunks at once)
    var_t = work.tile([128, NCH], F32, name=f"{name}_var", tag=f"{name}_var")
    nc.vector.tensor_mul(var_t, g8[:, :, 0], g8[:, :, 0])
    nc.vector.tensor_sub(var_t, g8[:, :, 1], var_t)
    nc.vector.tensor_scalar_add(var_t, var_t, EPS)
    # rstd = rsqrt(var): bit-trick seed + 2 Newton iterations (~fp32)
    rs = work.tile([128, NCH], F32, name=f"{name}_rs", tag=f"{name}_rs")
    vi = var_t.bitcast(mybir.dt.int32)
    ri = rs.bitcast(mybir.dt.int32)
    nc.vector.tensor_scalar(out=ri, in0=vi, scalar1=1, scalar2=None,
                            op0=mybir.AluOpType.arith_shift_right)
    nc.vector.tensor_scalar(out=ri, in0=ri, scalar1=-1, scalar2=0x5f3759df,
                            op0=mybir.AluOpType.mult,
                            op1=mybir.AluOpType.add)
    half = work.tile([128, NCH], F32, name=f"{name}_half", tag=f"{name}_half")
    nc.vector.tensor_scalar_mul(half, var_t, -0.5)
    tmp = work.tile([128, NCH], F32, name=f"{name}_tmp", tag=f"{name}_tmp")
    for _ in range(2):
        nc.vector.tensor_mul(tmp, rs, rs)
        nc.vector.tensor_mul(tmp, tmp, half)
        nc.vector.tensor_scalar_add(tmp, tmp, 1.5)
        nc.vector.tensor_mul(rs, rs, tmp)
    nc.vector.tensor_mul(scale_out, gam_sb, rs)
    # shift = beta - mean * scale
    nc.vector.tensor_mul(tmp, g8[:, :, 0], scale_out)
    nc.vector.tensor_sub(shift_out, bet_sb, tmp)


_COMPILED = None


def _get_compiled(a_slope: float):
    global _COMPILED
    if _COMPILED is None or _COMPILED[0] != a_slope:
        _COMPILED = (a_slope, _build(a_slope))
    return _COMPILED[1]


def _prep_inputs(inputs):
    x = np.ascontiguousarray(np.asarray(inputs["x"], dtype=np.float32))
    Wq = np.asarray(inputs["Wq"], dtype=np.float32)
    Wk = np.asarray(inputs["Wk"], dtype=np.float32)
    Wv = np.asarray(inputs["Wv"], dtype=np.float32)
    bq = np.asarray(inputs["bq"], dtype=np.float32)
    W1 = np.asarray(inputs["W1"], dtype=np.float32)
    b1 = np.asarray(inputs["b1"], dtype=np.float32)
    W2 = np.asarray(inputs["W2"], dtype=np.float32)
    gamma = np.asarray(inputs["gamma"], dtype=np.float32)
    beta = np.asarray(inputs["beta"], dtype=np.float32)

    def pack(mat):
        # [K, M] (K = contraction, rows grouped as (chunk, partition)) ->
        # [128, n_chunks * M] partition-major so the DMA is 128 big rows
        K, M = mat.shape
        return np.ascontiguousarray(
            mat.reshape(K // 128, 128, M).transpose(1, 0, 2).reshape(128, -1)
            .astype(np.float16))

    wvT = np.zeros((C, NH * DH), dtype=np.float32)
    for h in range(NH):
        wvT[:, h * DH:(h + 1) * DH] = Wv[h].T
    sm = np.zeros((128, 14), dtype=np.float32)
    sm[:, 0:NCH] = bq.reshape(NCH, 128).T
    sm[:, NCH:NCH + NFC] = b1.reshape(NFC, 128).T
    sm[:, NCH + NFC:NCH + NFC + NCH] = gamma.reshape(NCH, 128).T
    sm[:, NCH + NFC + NCH:] = beta.reshape(NCH, 128).T
    common = {
        "wqT": pack(Wq.reshape(C, C).T),
        "wkT": pack(Wk.reshape(C, C).T),
        "wvT": pack(wvT),
        "w1T": pack(W1.T),
        "w2T": pack(W2.T),
        "sm": sm,
    }
    # x: [B, C, N] -> per-core [BL, 128, NCH*N] partition-major f16
    xp = np.ascontiguousarray(
        x.reshape(B, NCH, 128, N).transpose(0, 2, 1, 3).reshape(B, 128, NCH * N)
        .astype(np.float16))
    in_maps = []
    for c in range(NCORES):
        m = dict(common)
        m["x"] = np.ascontiguousarray(xp[c * BL:(c + 1) * BL])
        in_maps.append(m)
    return in_maps


def kernel_ex(trace=False, **inputs):
    a_slope = float(np.asarray(inputs["a"]))
    nc = _get_compiled(a_slope)
    in_maps = _prep_inputs(inputs)
    res = bass_utils.run_bass_kernel_spmd(
        nc, in_maps, core_ids=list(range(NCORES)), trace=trace)
    out = np.empty((B, C, N), dtype=np.float32)
    for c in range(NCORES):
        out[c * BL:(c + 1) * BL] = res.results[c]["out"]
    return out.reshape(B, C, HH, WW), res


def kernel(**inputs):
    out, _ = kernel_ex(False, **inputs)
    return out


# revision 21
# speedup vs baseline: 1.0192x; 1.0192x over previous
"""Trainium2 Bass kernel for MultiHeadPosAttn (attention + BN + FFN + BN).

Sharding: data-parallel over batch across 8 NeuronCores (2 images/core).
BatchNorm batch statistics are combined with a tiny (2KB) AllReduce.

Math notes (verified exactly equivalent to the reference):
  - bk cancels in softmax (adds a per-query constant to every logit row).
  - bv cancels in BN1 (per-channel constant shift; softmax rows sum to 1).
  - b2 cancels in BN2 (per-channel constant shift).
  - PReLU(y) = Lrelu(y) with alpha = a (ACT supports a slope parameter).
  - softmax needs no max-subtraction: |logits| <= ~66 so exp() stays in
    fp32 range (max ~3e28 << 3.4e38).
Softmax denominator comes from an extra all-ones column in each head's
V^T block, so the attention matmul also produces sum_k(P) per query.
The V^T block for head h occupies lhsT columns so that the head's
output lands directly on its target partitions (even heads: d at
cols 0-63 + ones at col 64; odd heads: ones at col 63 + d at cols
64-127), avoiding any partition-shifting DMA.  The denominator row is
broadcast across partitions with a GPSIMD partition_broadcast (no DRAM
round trip).
"""

import numpy as np

import concourse.bass as bass
import concourse.bacc as bacc
import concourse.tile as tile
from concourse import mybir
from concourse import bass_utils

F32 = mybir.dt.float32
BF16 = mybir.dt.bfloat16
F16 = mybir.dt.float16

B, C, HH, WW = 16, 256, 32, 32
N = HH * WW              # 1024 spatial positions
NH, DH = 4, 64           # heads, head dim
DFF = 4 * C              # 1024
EPS = 1e-5
NCORES = 8
BL = B // NCORES         # 2 images per core
NCH = C // 128           # 2 channel chunks of 128
NFC = DFF // 128         # 8 ffn chunks
NNC = N // 128           # 8 position chunks


def _build(a_slope: float):
    nc = bacc.Bacc("TRN2", target_bir_lowering=False, debug=False,
                   num_devices=NCORES)

    # all big inputs are pre-packed host-side into [128, free] partition-major
    # layouts so every load DMA is 128 descriptors of >=512B contiguous rows
    x_d = nc.dram_tensor("x", [BL, 128, NCH * N], F16, kind="ExternalInput")
    wq_d = nc.dram_tensor("wqT", [128, NCH * C], F16, kind="ExternalInput")
    wk_d = nc.dram_tensor("wkT", [128, NCH * C], F16, kind="ExternalInput")
    wv_d = nc.dram_tensor("wvT", [128, NCH * NH * DH], F16, kind="ExternalInput")
    w1_d = nc.dram_tensor("w1T", [128, NCH * DFF], F16, kind="ExternalInput")
    w2_d = nc.dram_tensor("w2T", [128, NFC * C], F16, kind="ExternalInput")
    # bq(2) | b1(8) | gamma(2) | beta(2) packed per partition
    sm_d = nc.dram_tensor("sm", [128, 14], F32, kind="ExternalInput")
    out_d = nc.dram_tensor("out", [BL, C, N], F32, kind="ExternalOutput")

    with tile.TileContext(nc) as tc:
        _emit(tc, a_slope,
              x_d=x_d, wq_d=wq_d, wk_d=wk_d, wv_d=wv_d,
              w1_d=w1_d, w2_d=w2_d, sm_d=sm_d, out_d=out_d)
    nc.compile()
    return nc


def _emit(tc, a_slope, *, x_d, wq_d, wk_d, wv_d, w1_d, w2_d, sm_d, out_d):
    nc = tc.nc
    from contextlib import ExitStack

    ctx = ExitStack()
    with ctx:
        const = ctx.enter_context(tc.tile_pool(name="const", bufs=1))
        data = ctx.enter_context(tc.tile_pool(name="data", bufs=1))
        work = ctx.enter_context(tc.tile_pool(name="work", bufs=1))
        dram = ctx.enter_context(tc.tile_pool(name="dram", bufs=1, space="DRAM"))

        # ---- loads, spread across engines so the QKV-critical tensors
        # (wq, x0, wk, wv) land ASAP; FFN weights queue behind ----
        xs = []
        for img in range(BL):
            xs.append(data.tile([128, NCH, N], F16, name=f"xs{img}",
                                tag=f"xs{img}"))
        wq_sb = const.tile([128, NCH, C], F16, name="wq_sb")
        wk_sb = const.tile([128, NCH, C], F16, name="wk_sb")
        wv_sb = const.tile([128, NCH, NH * DH], F16, name="wv_sb")
        w1_sb = const.tile([128, NCH, DFF], F16, name="w1_sb")
        w2_sb = const.tile([128, NFC, C], F16, name="w2_sb")

        # scalar engine: wq then x0 (first Q matmul needs both), smalls
        nc.scalar.dma_start(out=wq_sb,
                            in_=wq_d.ap().rearrange("p (k m) -> p k m", m=C))
        nc.scalar.dma_start(out=xs[0],
                            in_=x_d.ap()[0].rearrange("p (c n) -> p c n", n=N))
        sm_sb = const.tile([128, 14], F32, name="sm_sb")
        nc.scalar.dma_start(out=sm_sb, in_=sm_d.ap())
        bq_sb = sm_sb[:, 0:NCH]
        b1_sb = sm_sb[:, NCH:NCH + NFC]
        gam_sb = sm_sb[:, NCH + NFC:NCH + NFC + NCH]
        bet_sb = sm_sb[:, NCH + NFC + NCH:NCH + NFC + 2 * NCH]
        # sync engine: wk, wv, x1
        nc.sync.dma_start(out=wk_sb,
                          in_=wk_d.ap().rearrange("p (k m) -> p k m", m=C))
        nc.sync.dma_start(out=wv_sb,
                          in_=wv_d.ap().rearrange("p (k m) -> p k m", m=NH * DH))
        nc.sync.dma_start(out=xs[1],
                          in_=x_d.ap()[1].rearrange("p (c n) -> p c n", n=N))
        # gpsimd: FFN weights (not needed until after attention)
        nc.gpsimd.dma_start(out=w1_sb,
                            in_=w1_d.ap().rearrange("p (k m) -> p k m", m=DFF))
        nc.gpsimd.dma_start(out=w2_sb,
                            in_=w2_d.ap().rearrange("p (k m) -> p k m", m=C))

        # PE warm-up: ~60 tiny matmuls straight after the preamble keep the
        # HAM activity window busy so QKV starts at 2.4GHz instead of 1.2.
        wrm_t = const.tile([128, 128], F16, name="wrm_t")
        nc.vector.memset(wrm_t, 0.5)
        warm_sb = const.tile([1, 64], F32, name="warm_sb")
        nc.vector.memset(warm_sb, 0.0)
        with tc.tile_pool(name="wrps", bufs=1, space="PSUM") as wrps:
            wp_t = wrps.tile([128, 128], F32, name="wp_t")
            for _ in range(60):
                nc.tensor.matmul(wp_t, lhsT=wrm_t, rhs=wrm_t,
                                 start=True, stop=True)
            # keep the dummies alive: route one lane into the warm payload
            nc.vector.tensor_copy(warm_sb[0:1, 63:64], wp_t[0:1, 0:1])
        for wi in range(1):
            w_in = dram.tile([64], F32, name=f"warm{wi}_in", tag=f"warm{wi}_in")
            w_out = dram.tile([64], F32, name=f"warm{wi}_out",
                              tag=f"warm{wi}_out", addr_space="Shared")
            nc.sync.dma_start(out=w_in.unsqueeze(0), in_=warm_sb)
            nc.gpsimd.collective_compute(
                "AllReduce", mybir.AluOpType.add,
                replica_groups=[list(range(NCORES))],
                ins=[w_in.opt()], outs=[w_out.opt()])

        # ---- persistent SBUF tensors ----
        q_sb, k_sb, vt_sb, o_sb, mh_sb, u_sb = [], [], [], [], [], []
        for img in range(BL):
            q_sb.append(data.tile([128, NCH, N], F16, name=f"q{img}", tag=f"q{img}"))
            k_sb.append(data.tile([128, NCH, N], F16, name=f"k{img}", tag=f"k{img}"))
            vt_sb.append(data.tile([128, NNC, NH * 128], BF16, name=f"vt{img}",
                                   tag=f"vt{img}"))
            o_sb.append(data.tile([128, NCH, N], F32, name=f"o{img}", tag=f"o{img}"))
            mh_sb.append(data.tile([128, NCH, N], F16, name=f"mh{img}",
                                   tag=f"mh{img}"))
            u_sb.append(data.tile([128, NCH, N], F32, name=f"u{img}", tag=f"u{img}"))

        # V^T layout per head block (128 cols): even heads [v(64) | 1 | 0*63],
        # odd heads [1 | 0*63 | v(64)] -- the ones (denominator) column must
        # land on a 32-aligned PSUM partition (0 or 64).
        for img in range(BL):
            vt4 = vt_sb[img].rearrange("p a (h d) -> p a h d", d=128)
            for h in range(NH):
                if h % 2 == 0:
                    nc.gpsimd.memset(vt4[:, :, h, DH + 1:128], 0.0)
                    nc.gpsimd.memset(vt4[:, :, h, DH:DH + 1], 1.0)
                else:
                    nc.gpsimd.memset(vt4[:, :, h, 1:DH], 0.0)
                    nc.gpsimd.memset(vt4[:, :, h, 0:1], 1.0)

        st1 = work.tile([128, NCH, BL * 2, 6], F32, name="bn1_stats",
                        tag="bn1_stats")
        st2 = work.tile([128, NCH, BL * 2, 6], F32, name="bn2_stats",
                        tag="bn2_stats")

        # =========== per image: QKV (own pools) then heads (own pools) ====
        def make_qkv(qkps, vtps):
            def emit_qkv_q(img):
                for mc in range(NCH):
                    qp = qkps.tile([128, N], F32, tag="qp", bufs=2)
                    for kc in range(NCH):
                        for mv in range(2):
                            nc.tensor.matmul(
                                qp[:, mv * 512:(mv + 1) * 512],
                                lhsT=(wq_sb[:, kc, mc * 128:(mc + 1) * 128]),
                                rhs=(xs[img][:, kc, mv * 512:(mv + 1) * 512]),
                                start=(kc == 0), stop=(kc == NCH - 1))
                    nc.scalar.activation(q_sb[img][:, mc, :], qp,
                                         mybir.ActivationFunctionType.Identity,
                                         bias=bq_sb[:, mc:mc + 1])

            def emit_qkv_k(img):
                for mc in range(NCH):
                    kp = qkps.tile([128, N], F32, tag="qp", bufs=2)
                    for kc in range(NCH):
                        for mv in range(2):
                            nc.tensor.matmul(
                                kp[:, mv * 512:(mv + 1) * 512],
                                lhsT=(wk_sb[:, kc, mc * 128:(mc + 1) * 128]),
                                rhs=(xs[img][:, kc, mv * 512:(mv + 1) * 512]),
                                start=(kc == 0), stop=(kc == NCH - 1))
                    nc.scalar.activation(k_sb[img][:, mc, :], kp,
                                         mybir.ActivationFunctionType.Identity)

            def emit_qkv_v(img, pcs):
                vt4 = vt_sb[img].rearrange("p a (h d) -> p a h d", d=128)
                for pc in pcs:
                    vp = vtps.tile([128, N], F32, tag="vp", bufs=2)
                    for kc in range(NCH):
                        nc.tensor.matmul(
                            vp[:, 0:NH * DH],
                            lhsT=(xs[img][:, kc, pc * 128:(pc + 1) * 128]),
                            rhs=(wv_sb[:, kc, :]),
                            start=(kc == 0), stop=(kc == NCH - 1))
                    for h in range(NH):
                        dst0 = 0 if h % 2 == 0 else 64
                        nc.vector.tensor_copy(
                            vt4[:, pc, h, dst0:dst0 + DH],
                            vp[:, h * DH:(h + 1) * DH])

            return emit_qkv_q, emit_qkv_k, emit_qkv_v

        def make_head(etps, oaps):
            def emit_head(img, h):
                hc, ho = h // 2, (h % 2) * 64
                denp = 64 if h % 2 == 0 else 0
                q_h = q_sb[img][ho:ho + 64, hc, :]
                k_h = k_sb[img][ho:ho + 64, hc, :]
                oaug = oaps.tile([128, N], F32, tag="oaug", bufs=2)
                for pc in range(NNC):
                    et = etps.tile([128, N], F32, tag="et", bufs=2)
                    for mv in range(2):
                        nc.tensor.matmul(
                            et[:, mv * 512:(mv + 1) * 512],
                            lhsT=(k_h[:, pc * 128:(pc + 1) * 128]),
                            rhs=(q_h[:, mv * 512:(mv + 1) * 512]),
                            start=True, stop=True)
                    p_t = work.tile([128, N], BF16, name="p_t", tag="p_t", bufs=6)
                    nc.scalar.activation(p_t, et,
                                         mybir.ActivationFunctionType.Exp)
                    for mv in range(2):
                        nc.tensor.matmul(
                            oaug[:, mv * 512:(mv + 1) * 512],
                            lhsT=(vt_sb[img][:, pc, h * 128:(h + 1) * 128]),
                            rhs=(p_t[:, mv * 512:(mv + 1) * 512]),
                            start=(pc == 0), stop=(pc == NNC - 1))
                # softmax denominator: row `denp` of oaug.  Copy to SBUF,
                # broadcast across all partitions on GPSIMD, reciprocal
                # (base-0 custom DVE op), then scale the head's 64 rows.
                # For the final head the chain is split into halves so the
                # BN1 stats (and the AllReduce behind them) start sooner.
                tail = (img == BL - 1 and h == NH - 1)
                halves = ((0, 512), (512, 1024)) if tail else ((0, 1024),)
                dsb = work.tile([128, N], F32, name="dsb", tag="dsb", bufs=2)
                dbc = work.tile([128, N], F32, name="dbc", tag="dbc", bufs=2)
                rbc = work.tile([128, N], F32, name="rbc", tag="rbc", bufs=2)
                dsb0 = None
                for lo, hi in halves:
                    nc.vector.tensor_copy(dsb[denp:denp + 1, lo:hi],
                                          oaug[denp:denp + 1, lo:hi])
                    srct = dsb
                    if denp != 0:
                        # partition_broadcast reads ABSOLUTE partition 0 on
                        # HW: bounce the row down with a small SBUF DMA.
                        if dsb0 is None:
                            dsb0 = work.tile([128, N], F32, name="dsb0",
                                             tag="dsb0", bufs=2)
                        nc.gpsimd.dma_start(out=dsb0[0:1, lo:hi],
                                            in_=dsb[denp:denp + 1, lo:hi])
                        srct = dsb0
                    nc.gpsimd.partition_broadcast(dbc[:, lo:hi],
                                                  srct[0:1, lo:hi])
                    nc.vector.reciprocal_approx_fast(out=rbc[:, lo:hi],
                                                     in_=dbc[:, lo:hi])
                    nc.vector.tensor_mul(o_sb[img][ho:ho + 64, hc, lo:hi],
                                         oaug[ho:ho + 64, lo:hi],
                                         rbc[ho:ho + 64, lo:hi])
                    if h % 2 == 1 and tail:
                        sg = lo // 512
                        nc.vector.tensor_add(
                            o_sb[img][:, hc, lo:hi],
                            o_sb[img][:, hc, lo:hi], xs[img][:, hc, lo:hi])
                        nc.vector.bn_stats(
                            out=st1[:, hc, img * 2 + sg, :],
                            in_=o_sb[img][:, hc, lo:hi])
                if h % 2 == 1 and not tail:
                    # both heads of chunk hc done -> residual + local stats
                    nc.vector.tensor_add(o_sb[img][:, hc, :],
                                         o_sb[img][:, hc, :],
                                         xs[img][:, hc, :])
                    for sg in range(2):
                        nc.vector.bn_stats(
                            out=st1[:, hc, img * 2 + sg, :],
                            in_=o_sb[img][:, hc, sg * 512:(sg + 1) * 512])

            return emit_head

        with tc.tile_pool(name="qkps", bufs=2, space="PSUM") as qkps, \
             tc.tile_pool(name="vtps", bufs=2, space="PSUM") as vtps:
            eq, ek, ev = make_qkv(qkps, vtps)
            for img in range(BL):
                eq(img)
                ek(img)
                ev(img, range(NNC))
        with tc.tile_pool(name="etps", bufs=2, space="PSUM") as etps, \
             tc.tile_pool(name="oaps", bufs=2, space="PSUM") as oaps:
            eh = make_head(etps, oaps)
            for img in range(BL):
                for h in range(NH):
                    eh(img, h)

        # =========== BN1 ===========
        s1_sb = work.tile([128, NCH], F32, name="s1_sb", tag="bns")
        t1_sb = work.tile([128, NCH], F32, name="t1_sb", tag="bnt")
        eps_unused = None
        cc1 = _bn_allreduce(tc, nc, work, dram, "bn1", st1)
        _bn_finish(tc, nc, work, "bn1", cc_out=cc1, gam_sb=gam_sb,
                   bet_sb=bet_sb, scale_out=s1_sb, shift_out=t1_sb)
        # apply: mh = s*(o+x) + t ; split across ACT (ch0) and DVE (ch1)
        for img in range(BL):
            nc.scalar.activation(mh_sb[img][:, 0, :], o_sb[img][:, 0, :],
                                 mybir.ActivationFunctionType.Identity,
                                 bias=t1_sb[:, 0:1], scale=s1_sb[:, 0:1])
            nc.vector.tensor_scalar(
                out=mh_sb[img][:, 1, :], in0=o_sb[img][:, 1, :],
                scalar1=s1_sb[:, 1:2], scalar2=t1_sb[:, 1:2],
                op0=mybir.AluOpType.mult, op1=mybir.AluOpType.add)

        # =========== FFN (mc-major W2 so stats start early) ===========
        ffs = [[work.tile([128, N], F16, name=f"ffs{img}_{fc}",
                          tag=f"ffs{fc}", bufs=2) for fc in range(NFC)]
               for img in range(BL)]
        with tc.tile_pool(name="ffps", bufs=2, space="PSUM") as ffps, \
             tc.tile_pool(name="ops2", bufs=2, space="PSUM") as ops2:
            def emit_w1(img, fc):
                # W1 matmul + PReLU for one ffn chunk (ACT Prelu honors
                # alpha; Lrelu ignores it on this HW)
                fp = ffps.tile([128, N], F32, tag="fp", bufs=2)
                for kc in range(NCH):
                    for mv in range(2):
                        nc.tensor.matmul(
                            fp[:, mv * 512:(mv + 1) * 512],
                            lhsT=(w1_sb[:, kc, fc * 128:(fc + 1) * 128]),
                            rhs=(mh_sb[img][:, kc, mv * 512:(mv + 1) * 512]),
                            start=(kc == 0), stop=(kc == NCH - 1))
                nc.scalar.activation(
                    ffs[img][fc], fp,
                    mybir.ActivationFunctionType.Prelu,
                    bias=b1_sb[:, fc:fc + 1], alpha=a_slope)

            for img in range(BL):
                for mc in range(NCH):
                    outp = ops2.tile([128, N], F32, tag="outp", bufs=2)
                    if mc == 0:
                        emit_w1(img, 0)
                    for fc in range(NFC):
                        # software pipeline: next chunk's W1 ahead of this
                        # chunk's W2 so PE never stalls on the Prelu
                        if mc == 0 and fc + 1 < NFC:
                            emit_w1(img, fc + 1)
                        for mv in range(2):
                            nc.tensor.matmul(
                                outp[:, mv * 512:(mv + 1) * 512],
                                lhsT=(w2_sb[:, fc, mc * 128:(mc + 1) * 128]),
                                rhs=(ffs[img][fc][:, mv * 512:(mv + 1) * 512]),
                                start=(fc == 0), stop=(fc == NFC - 1))
                    nc.vector.tensor_add(u_sb[img][:, mc, :], outp,
                                         mh_sb[img][:, mc, :])
                    for sg in range(2):
                        nc.vector.bn_stats(
                            out=st2[:, mc, img * 2 + sg, :],
                            in_=u_sb[img][:, mc, sg * 512:(sg + 1) * 512])

        # =========== BN2 + output ===========
        s2_sb = work.tile([128, NCH], F32, name="s2_sb", tag="bns2")
        t2_sb = work.tile([128, NCH], F32, name="t2_sb", tag="bnt2")
        cc2 = _bn_allreduce(tc, nc, work, dram, "bn2", st2)
        _bn_finish(tc, nc, work, "bn2", cc_out=cc2, gam_sb=gam_sb,
                   bet_sb=bet_sb, scale_out=s2_sb, shift_out=t2_sb)
        for img in range(BL):
            outr = out_d.ap()[img].rearrange("(c p) n -> p c n", p=128)
            # ch0 apply on ACT, ch1 on DVE, in half-chunks so the output DMAs
            # start as early as possible (o_sb is dead -> reuse as staging)
            for mv in range(2):
                sl = slice(mv * 512, (mv + 1) * 512)
                nc.scalar.activation(o_sb[img][:, 0, sl], u_sb[img][:, 0, sl],
                                     mybir.ActivationFunctionType.Identity,
                                     bias=t2_sb[:, 0:1], scale=s2_sb[:, 0:1])
                nc.sync.dma_start(out=outr[:, 0, sl], in_=o_sb[img][:, 0, sl])
                nc.vector.tensor_scalar(
                    out=o_sb[img][:, 1, sl], in0=u_sb[img][:, 1, sl],
                    scalar1=s2_sb[:, 1:2], scalar2=t2_sb[:, 1:2],
                    op0=mybir.AluOpType.mult, op1=mybir.AluOpType.add)
                nc.gpsimd.dma_start(out=outr[:, 1, sl], in_=o_sb[img][:, 1, sl])


def _bn_allreduce(tc, nc, work, dram, name, stats):
    """Aggregate all local bn_stats, pack [mean, msq], one 2KB AllReduce."""
    mv_t = work.tile([128, NCH, 2], F32, name=f"{name}_mv", tag=f"{name}_mv")
    pk = work.tile([128, NCH, 2], F32, name=f"{name}_pk", tag=f"{name}_pk")
    for ch in range(NCH):
        nc.vector.bn_aggr(out=mv_t[:, ch, :], in_=stats[:, ch, :, :])
    nc.vector.tensor_mul(pk[:, :, 0:1], mv_t[:, :, 0:1], mv_t[:, :, 0:1])
    nc.vector.tensor_add(pk[:, :, 1:2], mv_t[:, :, 1:2], pk[:, :, 0:1])
    nc.vector.tensor_copy(pk[:, :, 0:1], mv_t[:, :, 0:1])
    cc_in = dram.tile([128 * NCH * 2], F32, name=f"{name}_cc_in",
                      tag=f"{name}_cc_in")
    cc_out = dram.tile([128 * NCH * 2], F32, name=f"{name}_cc_out",
                       tag=f"{name}_cc_out", addr_space="Shared")
    nc.sync.dma_start(out=cc_in.rearrange("(p k) -> p k", p=128), in_=pk)
    nc.gpsimd.collective_compute(
        "AllReduce", mybir.AluOpType.add,
        replica_groups=[list(range(NCORES))],
        ins=[cc_in.opt()], outs=[cc_out.opt()])
    return cc_out


def _bn_finish(tc, nc, work, name, *, cc_out, gam_sb, bet_sb,
               scale_out, shift_out):
    """Turn the AllReduced [mean, msq] sums into per-channel scale/shift.
    rsqrt is DVE-only (bit-trick seed + Newton) to avoid an ACT table
    switch on the critical path."""
    sg_t = work.tile([128, NCH, 2], F32, name=f"{name}_sg", tag=f"{name}_sg")
    nc.sync.dma_start(out=sg_t, in_=cc_out.rearrange("(p k) -> p k", p=128))
    g8 = work.tile([128, NCH, 2], F32, name=f"{name}_g8", tag=f"{name}_g8")
    nc.vector.tensor_scalar_mul(g8, sg_t, 1.0 / NCORES)
    # var = msq - mean^2 + eps   (both channel chunks at once)
    var_t = work.tile([128, NCH], F32, name=f"{name}_var", tag=f"{name}_var")
    nc.vector.tensor_mul(var_t, g8[:, :, 0], g8[:, :, 0])
    nc.vector.tensor_sub(var_t, g8[:, :, 1], var_t)
    nc.vector.tensor_scalar_add(var_t, var_t, EPS)
    # rstd = rsqrt(var): bit-trick seed + 2 Newton iterations (~fp32)
    rs = work.tile([128, NCH], F32, name=f"{name}_rs", tag=f"{name}_rs")
    vi = var_t.bitcast(mybir.dt.int32)
    ri = rs.bitcast(mybir.dt.int32)
    nc.vector.tensor_scalar(out=ri, in0=vi, scalar1=1, scalar2=None,
                            op0=mybir.AluOpType.arith_shift_right)
    nc.vector.tensor_scalar(out=ri, in0=ri, scalar1=-1, scalar2=0x5f3759df,
                            op0=mybir.AluOpType.mult,
                            op1=mybir.AluOpType.add)
    half = work.tile([128, NCH], F32, name=f"{name}_half", tag=f"{name}_half")
    nc.vector.tensor_scalar_mul(half, var_t, -0.5)
    tmp = work.tile([128, NCH], F32, name=f"{name}_tmp", tag=f"{name}_tmp")
    for _ in range(2):
        nc.vector.tensor_mul(tmp, rs, rs)
        nc.vector.tensor_mul(tmp, tmp, half)
        nc.vector.tensor_scalar_add(tmp, tmp, 1.5)
        nc.vector.tensor_mul(rs, rs, tmp)
    nc.vector.tensor_mul(scale_out, gam_sb, rs)
    # shift = beta - mean * scale
    nc.vector.tensor_mul(tmp, g8[:, :, 0], scale_out)
    nc.vector.tensor_sub(shift_out, bet_sb, tmp)


_COMPILED = None


def _get_compiled(a_slope: float):
    global _COMPILED
    if _COMPILED is None or _COMPILED[0] != a_slope:
        _COMPILED = (a_slope, _build(a_slope))
    return _COMPILED[1]


def _prep_inputs(inputs):
    x = np.ascontiguousarray(np.asarray(inputs["x"], dtype=np.float32))
    Wq = np.asarray(inputs["Wq"], dtype=np.float32)
    Wk = np.asarray(inputs["Wk"], dtype=np.float32)
    Wv = np.asarray(inputs["Wv"], dtype=np.float32)
    bq = np.asarray(inputs["bq"], dtype=np.float32)
    W1 = np.asarray(inputs["W1"], dtype=np.float32)
    b1 = np.asarray(inputs["b1"], dtype=np.float32)
    W2 = np.asarray(inputs["W2"], dtype=np.float32)
    gamma = np.asarray(inputs["gamma"], dtype=np.float32)
    beta = np.asarray(inputs["beta"], dtype=np.float32)

    def pack(mat):
        # [K, M] (K = contraction, rows grouped as (chunk, partition)) ->
        # [128, n_chunks * M] partition-major so the DMA is 128 big rows
        K, M = mat.shape
        return np.ascontiguousarray(
            mat.reshape(K // 128, 128, M).transpose(1, 0, 2).reshape(128, -1)
            .astype(np.float16))

    wvT = np.zeros((C, NH * DH), dtype=np.float32)
    for h in range(NH):
        wvT[:, h * DH:(h + 1) * DH] = Wv[h].T
    sm = np.zeros((128, 14), dtype=np.float32)
    sm[:, 0:NCH] = bq.reshape(NCH, 128).T
    sm[:, NCH:NCH + NFC] = b1.reshape(NFC, 128).T
    sm[:, NCH + NFC:NCH + NFC + NCH] = gamma.reshape(NCH, 128).T
    sm[:, NCH + NFC + NCH:] = beta.reshape(NCH, 128).T
    common = {
        "wqT": pack(Wq.reshape(C, C).T),
        "wkT": pack(Wk.reshape(C, C).T),
        "wvT": pack(wvT),
        "w1T": pack(W1.T),
        "w2T": pack(W2.T),
        "sm": sm,
    }
    # x: [B, C, N] -> per-core [BL, 128, NCH*N] partition-major f16
    xp = np.ascontiguousarray(
        x.reshape(B, NCH, 128, N).transpose(0, 2, 1, 3).reshape(B, 128, NCH * N)
        .astype(np.float16))
    in_maps = []
    for c in range(NCORES):
        m = dict(common)
        m["x"] = np.ascontiguousarray(xp[c * BL:(c + 1) * BL])
        in_maps.append(m)
    return in_maps


def kernel_ex(trace=False, **inputs):
    a_slope = float(np.asarray(inputs["a"]))
    nc = _get_compiled(a_slope)
    in_maps = _prep_inputs(inputs)
    res = bass_utils.run_bass_kernel_spmd(
        nc, in_maps, core_ids=list(range(NCORES)), trace=trace)
    out = np.empty((B, C, N), dtype=np.float32)
    for c in range(NCORES):
        out[c * BL:(c + 1) * BL] = res.results[c]["out"]
    return out.reshape(B, C, HH, WW), res


def kernel(**inputs):
    out, _ = kernel_ex(False, **inputs)
    return out
